# revision 1
# baseline (speedup 1.0000x reference)
"""DiffusionBonds TRN2 Bass kernel (8 NeuronCores, edge-sharded).

Per-core plan (12500 real edges, padded to 12800 = 25 supertiles x 512):
  - indirect-gather packed [encoded|coords] rows for both bond endpoints
  - PE-transpose the gathered tiles to feature-major
  - layer1 factored: z1base[f,e] = W1a^T enc0T + W1b^T enc1T + w_dl (x) dl
    then 8 fused ACT ops r1[:,t,:] = lrelu(z1base + (t_j*w_t + b1))
  - layers 2/3 per t-chunk (matmul + fused lrelu+bias pass)
  - layer4 accumulated into stacked psum d16[(t,s), e] with zero-padded
    weight slices (c_s sign and b4 folded in)
  - PE-transpose d16 back to edge-major, one DVE op per r-block builds
    S[e, (r,s,t,k)] = d16c * dh
  - scatter-add DMA into partial[50048, 24] with host-precomputed
    globally-conflict-free dst indices; colliding records go to a trash
    row and are replayed from a DRAM scratch copy in a few spill waves.
Host: sums the 8 partials and adds `answer`.
"""
import sys

sys.path.insert(0, "/opt/trn_rl_repo")

import numpy as np

import concourse.bass as bass
import concourse.bacc as bacc_mod
import concourse.mybir as mybir
from concourse.tile import TileContext, add_dep_helper
from concourse.masks import make_identity
from concourse.bass_utils import run_bass_kernel_spmd

F32 = mybir.dt.float32
BF16 = mybir.dt.bfloat16
I32 = mybir.dt.int32

N, E, D, T = 50000, 100000, 128, 8
LEAKY = 0.001
NCORES = 8
EPC = E // NCORES          # 12500 real edges per core
ST = 512                   # edges per supertile
NST = 26                   # supertiles (25*512=12800 >= 12500 -> use 26? no: 25)
NST = 25
EC = ST * NST              # 12800 padded edges per core
RB = ST // 128             # 4 r-blocks per supertile
TRASH = N                  # trash row index in partial
PN = N + 48                # padded partial rows (>=N+1)
# spill wave capacities in columns of 128 records each
SPILL_CAPS = [64, 24, 10, 5, 3, 2, 1, 1]
SC = sum(SPILL_CAPS)       # 110 cols = 14080 record capacity


def build_kernel(debug=False):
    nc = bacc_mod.Bacc(trn_type="TRN2", name="diffbonds")

    table = nc.dram_tensor("table", [N, 132], F32, kind="ExternalInput")
    idx0 = nc.dram_tensor("idx0", [128, NST * RB], I32, kind="ExternalInput")
    idx1 = nc.dram_tensor("idx1", [128, NST * RB], I32, kind="ExternalInput")
    sidx = nc.dram_tensor("sidx", [128, NST * 2 * RB], I32, kind="ExternalInput")
    W1a = nc.dram_tensor("W1a", [128, 128], F32, kind="ExternalInput")
    W1b = nc.dram_tensor("W1b", [128, 128], F32, kind="ExternalInput")
    wt = nc.dram_tensor("wt", [128, 1], F32, kind="ExternalInput")
    wdl = nc.dram_tensor("wdl", [1, 128], F32, kind="ExternalInput")
    b1 = nc.dram_tensor("b1", [128, 1], F32, kind="ExternalInput")
    b2 = nc.dram_tensor("b2", [128, 1], F32, kind="ExternalInput")
    b3 = nc.dram_tensor("b3", [128, 1], F32, kind="ExternalInput")
    W2 = nc.dram_tensor("W2", [128, 128], F32, kind="ExternalInput")
    W3 = nc.dram_tensor("W3", [128, 128], F32, kind="ExternalInput")
    W4 = nc.dram_tensor("W4", [128, 2], F32, kind="ExternalInput")
    b4r = nc.dram_tensor("b4r", [1, 2], F32, kind="ExternalInput")
    tb = nc.dram_tensor("tb", [128, T], F32, kind="ExternalInput")

    partials = [nc.dram_tensor(f"partial{q}", [PN, 24], F32,
                               kind="ExternalOutput") for q in range(8)]
    if debug:
        dbg_z1 = nc.dram_tensor("dbg_z1", [128, ST], F32, kind="ExternalOutput")
        dbg_r1 = nc.dram_tensor("dbg_r1", [128, T, ST], F32, kind="ExternalOutput")
        dbg_d16 = nc.dram_tensor("dbg_d16", [16, ST], F32, kind="ExternalOutput")
        dbg_S = nc.dram_tensor("dbg_S", [128, RB, 2, 24], F32, kind="ExternalOutput")
        dbg_dh = nc.dram_tensor("dbg_dh", [128, RB, 3], F32, kind="ExternalOutput")
        dbg_g0 = nc.dram_tensor("dbg_g0", [128, RB, 132], F32, kind="ExternalOutput")

    with TileContext(nc) as tc:
        with tc.tile_pool(name="const", bufs=1) as cpool, \
             tc.tile_pool(name="gath", bufs=3) as gpool, \
             tc.tile_pool(name="enct", bufs=2) as epool, \
             tc.tile_pool(name="r1p", bufs=2) as r1pool, \
             tc.tile_pool(name="rxp", bufs=3) as rxpool, \
             tc.tile_pool(name="sp", bufs=2) as spool, \
             tc.tile_pool(name="geo", bufs=2) as geop, \
             tc.tile_pool(name="spill", bufs=2) as sppool, \
             tc.tile_pool(name="z1ps", bufs=1, space="PSUM") as z1psp, \
             tc.tile_pool(name="ckps", bufs=3, space="PSUM") as ckpsp, \
             tc.tile_pool(name="d16ps", bufs=2, space="PSUM") as d16psp, \
             tc.tile_pool(name="smps", bufs=2, space="PSUM") as smpsp, \
             tc.tile_pool(name="dscrap", bufs=1, space="DRAM") as dscr, \
             tc.tile_pool(name="dscratch", bufs=1, space="DRAM") as dsc2:

            # ---------------- constants / preloads ----------------
            w1a_f = cpool.tile([128, 128], F32)
            nc.sync.dma_start(out=w1a_f[:], in_=W1a[:, :])
            w1b_f = cpool.tile([128, 128], F32)
            nc.sync.dma_start(out=w1b_f[:], in_=W1b[:, :])
            w2_f = cpool.tile([128, 128], F32)
            nc.sync.dma_start(out=w2_f[:], in_=W2[:, :])
            w3_f = cpool.tile([128, 128], F32)
            nc.sync.dma_start(out=w3_f[:], in_=W3[:, :])
            w1a_t = cpool.tile([128, 128], BF16)
            nc.scalar.copy(out=w1a_t[:], in_=w1a_f[:])
            w1b_t = cpool.tile([128, 128], BF16)
            nc.scalar.copy(out=w1b_t[:], in_=w1b_f[:])
            w2_t = cpool.tile([128, 128], BF16)
            nc.scalar.copy(out=w2_t[:], in_=w2_f[:])
            w3_t = cpool.tile([128, 128], BF16)
            nc.scalar.copy(out=w3_t[:], in_=w3_f[:])
            w4_t = cpool.tile([128, 2], F32)
            nc.sync.dma_start(out=w4_t[:], in_=W4[:, :])
            b4r_t = cpool.tile([1, 2], F32)
            nc.sync.dma_start(out=b4r_t[:], in_=b4r[:, :])
            wt_t = cpool.tile([128, 1], F32)
            nc.sync.dma_start(out=wt_t[:], in_=wt[:, :])
            wdl_f = cpool.tile([1, 128], F32)
            nc.sync.dma_start(out=wdl_f[:], in_=wdl[:, :])
            wdl_t = cpool.tile([1, 128], BF16)
            nc.scalar.copy(out=wdl_t[:], in_=wdl_f[:])
            b1_t = cpool.tile([128, 1], F32)
            nc.sync.dma_start(out=b1_t[:], in_=b1[:, :])
            b2_t = cpool.tile([128, 1], F32)
            nc.sync.dma_start(out=b2_t[:], in_=b2[:, :])
            b3_t = cpool.tile([128, 1], F32)
            nc.sync.dma_start(out=b3_t[:], in_=b3[:, :])
            tb_t = cpool.tile([128, T], F32)
            nc.sync.dma_start(out=tb_t[:], in_=tb[:, :])
            idx0_t = cpool.tile([128, NST * RB], I32)
            nc.sync.dma_start(out=idx0_t[:], in_=idx0[:, :])
            idx1_t = cpool.tile([128, NST * RB], I32)
            nc.sync.dma_start(out=idx1_t[:], in_=idx1[:, :])
            sidx_t = cpool.tile([128, NST * 2 * RB], I32)
            nc.sync.dma_start(out=sidx_t[:], in_=sidx[:, :])

            ident = cpool.tile([128, 128], F32)
            make_identity(nc, ident[:])
            ones_t = cpool.tile([1, ST], BF16)
            nc.vector.memset(ones_t[:], 1.0)

            # W4cT[f, j*16 + (t*2+s)] = c_s*W4[f,s] if t==j else 0
            w4c_t = cpool.tile([128, T * 16], BF16)
            nc.vector.memset(w4c_t[:], 0.0)
            for j in range(T):
                nc.scalar.mul(out=w4c_t[:, j * 16 + j * 2: j * 16 + j * 2 + 1],
                              in_=w4_t[:, 0:1], mul=-0.5)
                nc.scalar.mul(out=w4c_t[:, j * 16 + j * 2 + 1: j * 16 + j * 2 + 2],
                              in_=w4_t[:, 1:2], mul=0.5)
            # b4c16[0, t*2+s] = c_s*b4[s]
            b4c_t = cpool.tile([1, 16], BF16)
            for s, c in ((0, -0.5), (1, 0.5)):
                nc.scalar.mul(
                    out=b4c_t[0:1, s:16:2],
                    in_=b4r_t[0:1, s:s + 1].to_broadcast([1, 8]),
                    mul=c)
            # cjs[f, j] = t[j]*w_t[f] + b1[f]
            cjs_t = cpool.tile([128, T], F32)
            nc.vector.tensor_tensor(out=cjs_t[:], in0=wt_t[:].to_broadcast([128, T]),
                                    in1=tb_t[:], op=mybir.AluOpType.mult)
            nc.vector.tensor_tensor(out=cjs_t[:], in0=cjs_t[:],
                                    in1=b1_t[:].to_broadcast([128, T]),
                                    op=mybir.AluOpType.add)

            scrap = dscr.tile([1, 4], I32)

            # Absorb the index-load completion sems into the Pool queue's
            # observed clock (indirect DMAs can carry only ONE sync wait).
            nc.gpsimd.dma_start(out=scrap[0:1, 0:1], in_=idx0_t[0:1, 0:1])
            nc.gpsimd.dma_start(out=scrap[0:1, 1:2], in_=idx1_t[0:1, 0:1])
            nc.gpsimd.dma_start(out=scrap[0:1, 2:3], in_=sidx_t[0:1, 0:1])

            # ---------------- main supertile loop ----------------
            # Gathers are emitted one supertile ahead so the in-order Q7
            # sequencer fills its scatter-dependency stalls with gather
            # emission (software pipelining on the Pool queue).
            def emit_gathers(st):
                g0 = gpool.tile([128, RB, 132], F32, tag="g0")
                for r in range(RB):
                    nc.gpsimd.indirect_dma_start(
                        out=g0[:, r, :], out_offset=None, in_=table[:],
                        in_offset=bass.IndirectOffsetOnAxis(
                            ap=idx0_t[:, st * RB + r:st * RB + r + 1], axis=0))
                g1 = gpool.tile([128, RB, 132], F32, tag="g1")
                for r in range(RB):
                    nc.gpsimd.indirect_dma_start(
                        out=g1[:, r, :], out_offset=None, in_=table[:],
                        in_offset=bass.IndirectOffsetOnAxis(
                            ap=idx1_t[:, st * RB + r:st * RB + r + 1], axis=0))
                return g0, g1

            prev_pe = None
            pend = [emit_gathers(0), emit_gathers(1)]
            for st in range(NST):
                if prev_pe is not None:
                    # absorb the PE tick (WAR: transposes read the g tiles)
                    # into the Pool queue so the next gathers carry <=1 wait
                    ab = nc.gpsimd.dma_start(out=scrap[0:1, 1:2],
                                             in_=ident[0:1, 0:1])
                    add_dep_helper(ab.ins, prev_pe.ins, sync=True,
                                   reason="absorb PE tick for gather WAR")
                g0, g1 = pend.pop(0)
                if st + 2 < NST:
                    pend.append(emit_gathers(st + 2))

                # transpose enc cols to feature-major
                encT0_ps = ckpsp.tile([128, ST], F32, tag="ck")
                for r in range(RB):
                    nc.tensor.transpose(out=encT0_ps[:, r * 128:(r + 1) * 128],
                                        in_=g0[:, r, 0:128], identity=ident[:])
                encT0 = epool.tile([128, ST], BF16, tag="e0")
                nc.vector.tensor_copy(out=encT0[:], in_=encT0_ps[:])
                encT1_ps = ckpsp.tile([128, ST], F32, tag="ck")
                for r in range(RB):
                    prev_pe = nc.tensor.transpose(
                        out=encT1_ps[:, r * 128:(r + 1) * 128],
                        in_=g1[:, r, 0:128], identity=ident[:])
                encT1 = epool.tile([128, ST], BF16, tag="e1")
                nc.vector.tensor_copy(out=encT1[:], in_=encT1_ps[:])

                # geometry (edge-major [128, RB, 3])
                dr = geop.tile([128, RB, 3], F32, tag="dr")
                nc.vector.tensor_tensor(out=dr[:], in0=g0[:, :, 128:131],
                                        in1=g1[:, :, 128:131],
                                        op=mybir.AluOpType.subtract)
                d2 = geop.tile([128, RB, 3], F32, tag="d2")
                nc.vector.tensor_tensor(out=d2[:], in0=dr[:], in1=dr[:],
                                        op=mybir.AluOpType.mult)
                dl2 = geop.tile([128, RB], F32, tag="dl2")
                nc.vector.tensor_reduce(out=dl2[:], in_=d2[:],
                                        op=mybir.AluOpType.add,
                                        axis=mybir.AxisListType.X)
                nc.vector.tensor_scalar_max(out=dl2[:], in0=dl2[:], scalar1=1e-12)
                dl = geop.tile([128, RB], F32, tag="dl")
                nc.scalar.sqrt(out=dl[:], in_=dl2[:])
                rdl = geop.tile([128, RB], F32, tag="rdl")
                nc.vector.reciprocal(out=rdl[:], in_=dl[:])
                dh = geop.tile([128, RB, 3], F32, tag="dh")
                nc.vector.tensor_tensor(out=dh[:], in0=dr[:],
                                        in1=rdl[:, :, None].to_broadcast([128, RB, 3]),
                                        op=mybir.AluOpType.mult)

                # dl flattened to a [1, ST] row for the K=1 rank-1 matmul
                dlT_ps = smpsp.tile([1, ST], F32, tag="sm")
                for r in range(RB):
                    nc.tensor.transpose(out=dlT_ps[0:1, r * 128:(r + 1) * 128],
                                        in_=dl[:, r:r + 1], identity=ident[:])
                dlT = geop.tile([1, ST], BF16, tag="dlT")
                nc.vector.tensor_copy(out=dlT[:], in_=dlT_ps[:])

                # layer 1 base (feature-major [128, ST])
                z1 = z1psp.tile([128, ST], F32, tag="z1")
                nc.tensor.matmul(out=z1[:], lhsT=w1a_t[:], rhs=encT0[:],
                                 start=True, stop=False)
                nc.tensor.matmul(out=z1[:], lhsT=w1b_t[:], rhs=encT1[:],
                                 start=False, stop=False)
                nc.tensor.matmul(out=z1[:], lhsT=wdl_t[0:1, :], rhs=dlT[0:1, :],
                                 start=False, stop=True)

                # expand over t with fused bias+lrelu
                r1 = r1pool.tile([128, T, ST], BF16, tag="r1")
                for j in range(T):
                    nc.scalar.activation(
                        out=r1[:, j, :], in_=z1[:],
                        func=mybir.ActivationFunctionType.Prelu,
                        bias=cjs_t[:, j:j + 1], scale=1.0, alpha=LEAKY)

                # layers 2..4 per t-chunk
                d16 = d16psp.tile([16, ST], F32, tag="d16")
                for j in range(T):
                    ps2 = ckpsp.tile([128, ST], F32, tag="ck")
                    nc.tensor.matmul(out=ps2[:], lhsT=w2_t[:], rhs=r1[:, j, :],
                                     start=True, stop=True)
                    r2 = rxpool.tile([128, ST], BF16, tag="r2")
                    nc.scalar.activation(
                        out=r2[:], in_=ps2[:],
                        func=mybir.ActivationFunctionType.Prelu,
                        bias=b2_t[:, 0:1], scale=1.0, alpha=LEAKY)
                    ps3 = ckpsp.tile([128, ST], F32, tag="ck")
                    nc.tensor.matmul(out=ps3[:], lhsT=w3_t[:], rhs=r2[:],
                                     start=True, stop=True)
                    r3 = rxpool.tile([128, ST], BF16, tag="r3")
                    nc.scalar.activation(
                        out=r3[:], in_=ps3[:],
                        func=mybir.ActivationFunctionType.Prelu,
                        bias=b3_t[:, 0:1], scale=1.0, alpha=LEAKY)
                    nc.tensor.matmul(out=d16[:], lhsT=w4c_t[:, j * 16:(j + 1) * 16],
                                     rhs=r3[:], start=(j == 0), stop=False)
                # bias c_s*b4[s] broadcast over edges
                nc.tensor.matmul(out=d16[:], lhsT=b4c_t[0:1, :], rhs=ones_t[0:1, :],
                                 start=False, stop=True)
                d16sb = geop.tile([16, ST], F32, tag="d16sb")
                nc.scalar.copy(out=d16sb[:], in_=d16[:])

                # back to edge-major and apply dh
                epiT = smpsp.tile([128, RB * 16], F32, tag="sm")
                for r in range(RB):
                    nc.tensor.transpose(out=epiT[:, r * 16:(r + 1) * 16],
                                        in_=d16sb[:, r * 128:(r + 1) * 128],
                                        identity=ident[0:16, 0:16])
                S = spool.tile([128, RB, 2, 24], F32, tag="S")
                for r in range(RB):
                    # in0: delta (t,s) -> order (s, t, k-bcast); in1: dh k
                    din = epiT[:, r * 16:(r + 1) * 16] \
                        .rearrange("p (t s) -> p s t", s=2)[:, :, :, None] \
                        .to_broadcast([128, 2, T, 3])
                    hin = dh[:, r, None, None, :].to_broadcast([128, 2, T, 3])
                    nc.vector.tensor_tensor(
                        out=S[:, r, :, :].rearrange("p s (t k) -> p s t k", k=3),
                        in0=din, in1=hin, op=mybir.AluOpType.mult)

                if debug and st == 0:
                    z1sb_d = geop.tile([128, ST], F32, tag="z1d")
                    nc.vector.tensor_copy(out=z1sb_d[:], in_=z1[:])
                    nc.sync.dma_start(out=dbg_z1[:, :], in_=z1sb_d[:])
                    nc.sync.dma_start(out=dbg_r1[:, :, :], in_=r1[:])
                    nc.sync.dma_start(out=dbg_d16[:, :], in_=d16sb[:])
                    nc.sync.dma_start(out=dbg_S[:, :, :, :], in_=S[:])
                    nc.sync.dma_start(out=dbg_dh[:, :, :], in_=dh[:])
                    nc.sync.dma_start(out=dbg_g0[:, :, :], in_=g0[:])

                # absorb the DVE (S producer) sem into the Pool queue, then
                # 8 single-row scatter-adds (multi-row offset APs are broken).
                # Batches are intra-conflict-free by host edge-block coloring;
                # cross-batch conflicts are ordered by Tile's WAW serialization.
                nc.gpsimd.dma_start(out=scrap[0:1, 1:2], in_=S[0:1, 0, 0, 0:1])
                for rs in range(2 * RB):
                    nc.gpsimd.indirect_dma_start(
                        out=partials[rs][:],
                        out_offset=bass.IndirectOffsetOnAxis(
                            ap=sidx_t[:, st * 2 * RB + rs:st * 2 * RB + rs + 1],
                            axis=0),
                        in_=S[:, rs // 2, rs % 2, :],
                        in_offset=None,
                        compute_op=mybir.AluOpType.add)

    nc.finalize()
    return nc


# ---------------------------------------------------------------------------
# host-side sharding / index preparation
# ---------------------------------------------------------------------------

def _prep_core_inputs(bonds_shard, table, consts):
    """Assign edges to 128-slot blocks s.t. within each block all i0 are
    distinct and all i1 are distinct (scatter batches conflict-free)."""
    nreal = bonds_shard.shape[0]
    nblocks = EC // 128
    seen0 = [set() for _ in range(nblocks)]
    seen1 = [set() for _ in range(nblocks)]
    fill = np.zeros(nblocks, np.int32)
    slot_i0 = np.zeros((nblocks, 128), np.int32)
    slot_i1 = np.zeros((nblocks, 128), np.int32)
    slot_real = np.zeros((nblocks, 128), bool)
    for jj in range(nreal):
        a, b = int(bonds_shard[jj, 0]), int(bonds_shard[jj, 1])
        bi = 0
        while True:
            assert bi < nblocks, "edge placement failed"
            if fill[bi] < 128 and a not in seen0[bi] and b not in seen1[bi]:
                p = fill[bi]
                fill[bi] += 1
                seen0[bi].add(a)
                seen1[bi].add(b)
                slot_i0[bi, p] = a
                slot_i1[bi, p] = b
                slot_real[bi, p] = True
                break
            bi += 1

    # block bb -> (st, r);  idx cols st*RB + r;  sidx cols st*2*RB + r*2 + s
    idx0 = np.zeros((128, NST * RB), np.int32)
    idx1 = np.zeros((128, NST * RB), np.int32)
    sidx = np.full((128, NST * 2 * RB), TRASH, np.int32)
    for bb in range(nblocks):
        st, r = bb // RB, bb % RB
        idx0[:, st * RB + r] = slot_i0[bb]
        idx1[:, st * RB + r] = slot_i1[bb]
        real = slot_real[bb]
        c0 = st * 2 * RB + r * 2
        sidx[real, c0] = slot_i0[bb][real]
        sidx[real, c0 + 1] = slot_i1[bb][real]

    inp = dict(table=table, idx0=idx0, idx1=idx1, sidx=sidx)
    inp.update(consts)
    return inp


def _run(in_maps, trace=False, debug=False):
    nc = build_kernel(debug=debug)
    kw = {}
    if trace:
        kw = dict(trace=True, trace_cores=[0])
    return run_bass_kernel_spmd(nc, in_maps, core_ids=list(range(NCORES)), **kw)


def kernel(coords, encoded, t, answer, W1, b1, W2, b2, W3, b3, W4, b4, bonds):
    coords = np.asarray(coords, np.float32)
    encoded = np.asarray(encoded, np.float32)
    t = np.asarray(t, np.float32)
    answer = np.asarray(answer, np.float32)
    W1 = np.asarray(W1, np.float32)
    W2 = np.asarray(W2, np.float32)
    W3 = np.asarray(W3, np.float32)
    W4 = np.asarray(W4, np.float32)
    b1 = np.asarray(b1, np.float32)
    b2 = np.asarray(b2, np.float32)
    b3 = np.asarray(b3, np.float32)
    b4 = np.asarray(b4, np.float32)
    bonds = np.asarray(bonds)

    table = np.concatenate(
        [encoded, coords, np.zeros((N, 1), np.float32)], axis=1)
    table = np.ascontiguousarray(table, np.float32)

    consts = dict(
        W1a=np.ascontiguousarray(W1[0:128, :]),
        W1b=np.ascontiguousarray(W1[128:256, :]),
        wt=np.ascontiguousarray(W1[256, :].reshape(128, 1)),
        wdl=np.ascontiguousarray(W1[257, :].reshape(1, 128)),
        b1=b1.reshape(128, 1).copy(),
        b2=b2.reshape(128, 1).copy(),
        b3=b3.reshape(128, 1).copy(),
        W2=np.ascontiguousarray(W2),
        W3=np.ascontiguousarray(W3),
        W4=np.ascontiguousarray(W4),
        b4r=b4.reshape(1, 2).copy(),
        tb=np.ascontiguousarray(np.broadcast_to(t, (128, T))),
    )

    in_maps = []
    for c in range(NCORES):
        shard = bonds[c * EPC:(c + 1) * EPC]
        in_maps.append(_prep_core_inputs(shard, table, consts))

    res = _run(in_maps)

    out = answer.reshape(N, T * 3).astype(np.float32).copy()
    for c in range(NCORES):
        for q in range(8):
            out += res.results[c][f"partial{q}"][:N]
    return out.reshape(N, T, 3)


def kernel_traced(coords, encoded, t, answer, W1, b1, W2, b2, W3, b3, W4, b4,
                  bonds):
    """Like kernel() but captures an NTFF profile; returns (out, exec_ns)."""
    coords = np.asarray(coords, np.float32)
    encoded = np.asarray(encoded, np.float32)
    t = np.asarray(t, np.float32)
    answer = np.asarray(answer, np.float32)
    table = np.concatenate(
        [encoded, coords, np.zeros((N, 1), np.float32)], axis=1)
    table = np.ascontiguousarray(table, np.float32)
    W1 = np.asarray(W1, np.float32)
    consts = dict(
        W1a=np.ascontiguousarray(W1[0:128, :]),
        W1b=np.ascontiguousarray(W1[128:256, :]),
        wt=np.ascontiguousarray(W1[256, :].reshape(128, 1)),
        wdl=np.ascontiguousarray(W1[257, :].reshape(1, 128)),
        b1=np.asarray(b1, np.float32).reshape(128, 1).copy(),
        b2=np.asarray(b2, np.float32).reshape(128, 1).copy(),
        b3=np.asarray(b3, np.float32).reshape(128, 1).copy(),
        W2=np.ascontiguousarray(np.asarray(W2, np.float32)),
        W3=np.ascontiguousarray(np.asarray(W3, np.float32)),
        W4=np.ascontiguousarray(np.asarray(W4, np.float32)),
        b4r=np.asarray(b4, np.float32).reshape(1, 2).copy(),
        tb=np.ascontiguousarray(np.broadcast_to(t, (128, T))),
    )
    bonds = np.asarray(bonds)
    in_maps = []
    for c in range(NCORES):
        shard = bonds[c * EPC:(c + 1) * EPC]
        in_maps.append(_prep_core_inputs(shard, table, consts))

    res = _run(in_maps, trace=True)

    out = answer.reshape(N, T * 3).astype(np.float32).copy()
    for c in range(NCORES):
        for q in range(8):
            out += res.results[c][f"partial{q}"][:N]
    return out.reshape(N, T, 3), res.exec_time_ns


if __name__ == "__main__":
    # smoke: build only
    nc = build_kernel()
    print("built ok")



# revision 2
# speedup vs baseline: 1.3592x; 1.3592x over previous
"""DiffusionBonds TRN2 Bass kernel v2 (8 NeuronCores, edge-sharded).

Per-core plan (12500 real edges, 25 supertiles x 512):
  - per-core atom renumbering (~19.7K unique atoms < 2^15) so the batched
    int16-indexed dma_gather / dma_scatter_add instructions apply
  - ONE dma_gather (transpose=True, bf16) per supertile fetches both
    endpoints' encoded rows FEATURE-MAJOR [128, 1024] -> feeds matmuls
    directly (no PE transposes for enc)
  - coords streamed densely as host-packed edge pairs; geometry (dr, dl,
    dh) computed on DVE/scalar
  - layer1 factored: z1 = W1a^T enc0T + W1b^T enc1T + wdl (x) dlT; t
    expansion fused with bias+lrelu (split scalar ACT / DVE chains)
  - layers 2/3 phase-ordered (all t per layer -> one LDWEIGHTS each),
    activations split between scalar ACT (fused bias prelu) and DVE
    (tensor_scalar chains)
  - layer4 accumulated into stacked psum d16[(t,s), e] with zero-padded
    c_s-signed weight slices; bias folded into the psum->sbuf Identity
    activation; PE-transpose back to edge-major; DVE builds
    S[e,(r,s,t,k)] = d16c * dh
  - ONE dma_scatter_add per supertile (1024 records); host coloring
    guarantees each atom appears at most once per supertile (duplicate
    indices lose updates in one instruction); consecutive supertiles
    round-robin over KP partial tensors, Tile WAW serializes same-tensor
    scatters
Host: per-core local partial sums mapped back through the atom
renumbering and added to `answer`.
"""
import sys

sys.path.insert(0, "/opt/trn_rl_repo")

import numpy as np
import ml_dtypes

import concourse.bass as bass
import concourse.bacc as bacc_mod
import concourse.mybir as mybir
from concourse.tile import TileContext, add_dep_helper
from concourse.masks import make_identity
from concourse.bass_utils import run_bass_kernel_spmd

F32 = mybir.dt.float32
BF16 = mybir.dt.bfloat16
I16 = mybir.dt.int16

N, E, D, T = 50000, 100000, 128, 8
LEAKY = 0.001
NCORES = 8
EPC = E // NCORES          # 12500 real edges per core
ST = 512                   # edges per supertile
NST = 25                   # supertiles
EC = ST * NST              # 12800 padded edges per core
RB = ST // 128             # 4 r-blocks per supertile
UPAD = 21504               # local atom table rows (unique ~19.7K)
TRASH = UPAD - 1           # trash row for pad/self-loop records
KP = 4                     # round-robin partial tensors
GCOL = 2 * ST // 16        # idx cols per supertile (64)

# activation engine split: which t-chunks go on scalar ACT (rest on DVE)
import os
CONSERVATIVE = os.environ.get("K2_CONSERVATIVE", "0") == "1"
if CONSERVATIVE:
    TEXP_SC = 8
    L2_SC = tuple(range(8))
    L3_SC = tuple(range(8))
else:
    TEXP_SC = 4            # t-exp: j < TEXP_SC on scalar, rest DVE batched
    L2_SC = (0, 2, 4, 6)   # layer2 acts on scalar
    L3_SC = (0, 2, 4, 6)   # layer3 acts on scalar


def build_kernel():
    nc = bacc_mod.Bacc(trn_type="TRN2", name="diffbonds2")

    tabL = nc.dram_tensor("tabL", [UPAD, 128], BF16, kind="ExternalInput")
    gidx = nc.dram_tensor("gidx", [128, NST * GCOL], I16, kind="ExternalInput")
    sidx = nc.dram_tensor("sidx", [128, NST * GCOL], I16, kind="ExternalInput")
    cpair = nc.dram_tensor("cpair", [128, NST * RB * 6], F32,
                           kind="ExternalInput")
    W1a = nc.dram_tensor("W1a", [128, 128], F32, kind="ExternalInput")
    W1b = nc.dram_tensor("W1b", [128, 128], F32, kind="ExternalInput")
    wt = nc.dram_tensor("wt", [128, 1], F32, kind="ExternalInput")
    wdl = nc.dram_tensor("wdl", [1, 128], F32, kind="ExternalInput")
    b1 = nc.dram_tensor("b1", [128, 1], F32, kind="ExternalInput")
    b2 = nc.dram_tensor("b2", [128, 1], F32, kind="ExternalInput")
    b3 = nc.dram_tensor("b3", [128, 1], F32, kind="ExternalInput")
    W2 = nc.dram_tensor("W2", [128, 128], F32, kind="ExternalInput")
    W3 = nc.dram_tensor("W3", [128, 128], F32, kind="ExternalInput")
    W4 = nc.dram_tensor("W4", [128, 2], F32, kind="ExternalInput")
    b4c = nc.dram_tensor("b4c", [16, 1], F32, kind="ExternalInput")
    tb = nc.dram_tensor("tb", [128, T], F32, kind="ExternalInput")

    partials = [nc.dram_tensor(f"partial{q}", [UPAD, 64], F32,
                               kind="ExternalOutput") for q in range(KP)]

    with TileContext(nc) as tc:
        with tc.tile_pool(name="const", bufs=1) as cpool, \
             tc.tile_pool(name="gath", bufs=3) as gpool, \
             tc.tile_pool(name="r1p", bufs=2) as r1pool, \
             tc.tile_pool(name="r2p", bufs=2) as r2pool, \
             tc.tile_pool(name="r3p", bufs=2) as r3pool, \
             tc.tile_pool(name="actt", bufs=3) as apool, \
             tc.tile_pool(name="sp", bufs=4) as spool, \
             tc.tile_pool(name="geo", bufs=2) as geop, \
             tc.tile_pool(name="z1ps", bufs=1, space="PSUM") as z1psp, \
             tc.tile_pool(name="ckps", bufs=3, space="PSUM") as ckpsp, \
             tc.tile_pool(name="d16ps", bufs=2, space="PSUM") as d16psp, \
             tc.tile_pool(name="smps", bufs=2, space="PSUM") as smpsp, \
             tc.tile_pool(name="dscrap", bufs=1, space="DRAM") as dscr:

            # ---------------- constants / preloads ----------------
            def load_cast_bf16(name, src, shape):
                f = cpool.tile(shape, F32, tag=f"{name}_f")
                nc.sync.dma_start(out=f[:], in_=src[:, :])
                b = cpool.tile(shape, BF16, tag=f"{name}_b")
                nc.scalar.copy(out=b[:], in_=f[:])
                return f, b

            w1a_f, w1a_t = load_cast_bf16("w1a", W1a, [128, 128])
            w1b_f, w1b_t = load_cast_bf16("w1b", W1b, [128, 128])
            w2_f, w2_t = load_cast_bf16("w2", W2, [128, 128])
            w3_f, w3_t = load_cast_bf16("w3", W3, [128, 128])
            wdl_f, wdl_t = load_cast_bf16("wdl", wdl, [1, 128])

            w4_t = cpool.tile([128, 2], F32)
            nc.sync.dma_start(out=w4_t[:], in_=W4[:, :])
            wt_t = cpool.tile([128, 1], F32)
            nc.sync.dma_start(out=wt_t[:], in_=wt[:, :])
            b1_t = cpool.tile([128, 1], F32)
            nc.sync.dma_start(out=b1_t[:], in_=b1[:, :])
            b2_t = cpool.tile([128, 1], F32)
            nc.sync.dma_start(out=b2_t[:], in_=b2[:, :])
            b3_t = cpool.tile([128, 1], F32)
            nc.sync.dma_start(out=b3_t[:], in_=b3[:, :])
            b4c_t = cpool.tile([16, 1], F32)
            nc.sync.dma_start(out=b4c_t[:], in_=b4c[:, :])
            tb_t = cpool.tile([128, T], F32)
            nc.sync.dma_start(out=tb_t[:], in_=tb[:, :])
            gidx_t = cpool.tile([128, NST * GCOL], I16)
            nc.sync.dma_start(out=gidx_t[:], in_=gidx[:, :])
            sidx_t = cpool.tile([128, NST * GCOL], I16)
            nc.sync.dma_start(out=sidx_t[:], in_=sidx[:, :])
            cpair_t = cpool.tile([128, NST * RB, 6], F32)
            nc.sync.dma_start(
                out=cpair_t[:],
                in_=cpair[:, :].rearrange("p (r k) -> p r k", k=6))

            ident = cpool.tile([128, 128], F32)
            make_identity(nc, ident[:])



            # W4cT[f, j*16 + (t*2+s)] = c_s*W4[f,s] if t==j else 0
            w4c_t = cpool.tile([128, T * 16], BF16)
            nc.vector.memset(w4c_t[:], 0.0)
            for j in range(T):
                nc.scalar.mul(out=w4c_t[:, j * 16 + j * 2: j * 16 + j * 2 + 1],
                              in_=w4_t[:, 0:1], mul=-0.5)
                nc.scalar.mul(out=w4c_t[:, j * 16 + j * 2 + 1: j * 16 + j * 2 + 2],
                              in_=w4_t[:, 1:2], mul=0.5)
            # cjs[f, j] = t[j]*w_t[f] + b1[f]
            cjs_t = cpool.tile([128, T], F32)
            nc.vector.tensor_tensor(out=cjs_t[:], in0=wt_t[:].to_broadcast([128, T]),
                                    in1=tb_t[:], op=mybir.AluOpType.mult)
            nc.vector.tensor_tensor(out=cjs_t[:], in0=cjs_t[:],
                                    in1=b1_t[:].to_broadcast([128, T]),
                                    op=mybir.AluOpType.add)

            scrap = dscr.tile([1, 4], mybir.dt.int32)
            gt_last_reader = []

            # ---------------- main supertile loop ----------------
            for st in range(NST):
                if st >= 3:
                    # absorb the reused gt buffer's WAR tick into the Pool
                    # queue clock so the gather carries <= 1 sync wait
                    ab = nc.gpsimd.dma_start(out=scrap[0:1, 0:1],
                                             in_=gidx_t[0:1, 0:1])
                    add_dep_helper(ab.ins, gt_last_reader[st - 3].ins,
                                   sync=True, reason="absorb gt WAR")
                # batched indexed gathers (512 idxs each, probe-validated
                # granularity): endpoint 0 then endpoint 1, feature-major bf16
                gt = gpool.tile([128, 1, 2 * ST], BF16, tag="gt")
                for half in range(2):
                    nc.gpsimd.dma_gather(
                        out_ap=gt[:, 0:1, half * ST:(half + 1) * ST],
                        in_ap=tabL[:, :],
                        idxs_ap=gidx_t[:, st * GCOL + half * (GCOL // 2):
                                       st * GCOL + (half + 1) * (GCOL // 2)],
                        num_idxs=ST,
                        num_idxs_reg=ST,
                        elem_size=128,
                        transpose=True,
                    )

                # geometry (edge-major [128, RB, 3])
                cp = cpair_t[:, st * RB:(st + 1) * RB, :]
                dr = geop.tile([128, RB, 3], F32, tag="dr")
                nc.vector.tensor_tensor(out=dr[:], in0=cp[:, :, 0:3],
                                        in1=cp[:, :, 3:6],
                                        op=mybir.AluOpType.subtract)
                d2 = geop.tile([128, RB, 3], F32, tag="d2")
                nc.vector.tensor_tensor(out=d2[:], in0=dr[:], in1=dr[:],
                                        op=mybir.AluOpType.mult)
                dl2 = geop.tile([128, RB], F32, tag="dl2")
                nc.vector.tensor_reduce(out=dl2[:], in_=d2[:],
                                        op=mybir.AluOpType.add,
                                        axis=mybir.AxisListType.X)
                nc.vector.tensor_scalar_max(out=dl2[:], in0=dl2[:], scalar1=1e-12)
                dl = geop.tile([128, RB], F32, tag="dl")
                nc.scalar.sqrt(out=dl[:], in_=dl2[:])
                rdl = geop.tile([128, RB], F32, tag="rdl")
                nc.vector.reciprocal(out=rdl[:], in_=dl[:])
                dh = geop.tile([128, RB, 3], F32, tag="dh")
                nc.vector.tensor_tensor(out=dh[:], in0=dr[:],
                                        in1=rdl[:, :, None].to_broadcast([128, RB, 3]),
                                        op=mybir.AluOpType.mult)

                # dl flattened to a [1, ST] bf16 row for the rank-1 matmul
                dlT_ps = smpsp.tile([1, ST], F32, tag="sm")
                for r in range(RB):
                    nc.tensor.transpose(out=dlT_ps[0:1, r * 128:(r + 1) * 128],
                                        in_=dl[:, r:r + 1], identity=ident[:])
                dlT = geop.tile([1, ST], BF16, tag="dlT")
                nc.vector.tensor_copy(out=dlT[:], in_=dlT_ps[:])

                # layer 1 base (feature-major [128, ST])
                z1 = z1psp.tile([128, ST], F32, tag="z1")
                nc.tensor.matmul(out=z1[:], lhsT=w1a_t[:], rhs=gt[:, 0, 0:ST],
                                 start=True, stop=False)
                mmb = nc.tensor.matmul(out=z1[:], lhsT=w1b_t[:],
                                       rhs=gt[:, 0, ST:2 * ST],
                                       start=False, stop=False)
                gt_last_reader.append(mmb)
                nc.tensor.matmul(out=z1[:], lhsT=wdl_t[0:1, :], rhs=dlT[0:1, :],
                                 start=False, stop=True)

                # t-expansion with bias+lrelu
                r1 = r1pool.tile([128, T, ST], BF16, tag="r1")
                for j in range(TEXP_SC):
                    nc.scalar.activation(
                        out=r1[:, j, :], in_=z1[:],
                        func=mybir.ActivationFunctionType.Prelu,
                        bias=cjs_t[:, j:j + 1], scale=1.0, alpha=LEAKY)
                ntd = T - TEXP_SC
                if ntd:
                    z1sb = apool.tile([128, ST], BF16, tag="z1sb")
                    nc.vector.tensor_copy(out=z1sb[:], in_=z1[:])
                    u8 = apool.tile([128, ntd, ST], BF16, tag="u8")
                    nc.vector.tensor_tensor(
                        out=u8[:],
                        in0=z1sb[:, None, :].to_broadcast([128, ntd, ST]),
                        in1=cjs_t[:, TEXP_SC:T, None].to_broadcast([128, ntd, ST]),
                        op=mybir.AluOpType.add)
                    m8 = apool.tile([128, ntd, ST], BF16, tag="m8")
                    nc.vector.tensor_scalar_mul(out=m8[:], in0=u8[:],
                                                scalar1=LEAKY)
                    nc.vector.tensor_tensor(out=r1[:, TEXP_SC:T, :], in0=u8[:],
                                            in1=m8[:], op=mybir.AluOpType.max)

                # ---- layer 2 (phase ordered) ----
                r2 = r2pool.tile([128, T, ST], BF16, tag="r2")
                for j in range(T):
                    ps = ckpsp.tile([128, ST], F32, tag="ck")
                    nc.tensor.matmul(out=ps[:], lhsT=w2_t[:], rhs=r1[:, j, :],
                                     start=True, stop=True)
                    if j in L2_SC:
                        nc.scalar.activation(
                            out=r2[:, j, :], in_=ps[:],
                            func=mybir.ActivationFunctionType.Prelu,
                            bias=b2_t[:, 0:1], scale=1.0, alpha=LEAKY)
                    else:
                        uj = apool.tile([128, ST], BF16, tag="uj2")
                        nc.vector.tensor_scalar_add(out=uj[:], in0=ps[:],
                                                    scalar1=b2_t[:, 0:1])
                        mj = apool.tile([128, ST], BF16, tag="mj2")
                        nc.vector.tensor_scalar(out=mj[:], in0=ps[:],
                                                scalar1=b2_t[:, 0:1],
                                                scalar2=LEAKY,
                                                op0=mybir.AluOpType.add,
                                                op1=mybir.AluOpType.mult)
                        nc.vector.tensor_tensor(out=r2[:, j, :], in0=uj[:],
                                                in1=mj[:],
                                                op=mybir.AluOpType.max)

                # ---- layer 3 ----
                r3 = r3pool.tile([128, T, ST], BF16, tag="r3")
                for j in range(T):
                    ps = ckpsp.tile([128, ST], F32, tag="ck")
                    nc.tensor.matmul(out=ps[:], lhsT=w3_t[:], rhs=r2[:, j, :],
                                     start=True, stop=True)
                    if j in L3_SC:
                        nc.scalar.activation(
                            out=r3[:, j, :], in_=ps[:],
                            func=mybir.ActivationFunctionType.Prelu,
                            bias=b3_t[:, 0:1], scale=1.0, alpha=LEAKY)
                    else:
                        uj = apool.tile([128, ST], BF16, tag="uj3")
                        nc.vector.tensor_scalar_add(out=uj[:], in0=ps[:],
                                                    scalar1=b3_t[:, 0:1])
                        mj = apool.tile([128, ST], BF16, tag="mj3")
                        nc.vector.tensor_scalar(out=mj[:], in0=ps[:],
                                                scalar1=b3_t[:, 0:1],
                                                scalar2=LEAKY,
                                                op0=mybir.AluOpType.add,
                                                op1=mybir.AluOpType.mult)
                        nc.vector.tensor_tensor(out=r3[:, j, :], in0=uj[:],
                                                in1=mj[:],
                                                op=mybir.AluOpType.max)

                # ---- layer 4 stacked ----
                d16 = d16psp.tile([16, ST], F32, tag="d16")
                for j in range(T):
                    nc.tensor.matmul(out=d16[:], lhsT=w4c_t[:, j * 16:(j + 1) * 16],
                                     rhs=r3[:, j, :], start=(j == 0),
                                     stop=(j == T - 1))
                # bias c_s*b4[s] folded into the psum->sbuf drain
                d16sb = geop.tile([16, ST], F32, tag="d16sb")
                nc.scalar.activation(out=d16sb[:], in_=d16[:],
                                     func=mybir.ActivationFunctionType.Identity,
                                     bias=b4c_t[:, 0:1], scale=1.0)

                # back to edge-major and apply dh
                epiT = smpsp.tile([128, RB * 16], F32, tag="sm")
                for r in range(RB):
                    nc.tensor.transpose(out=epiT[:, r * 16:(r + 1) * 16],
                                        in_=d16sb[:, r * 128:(r + 1) * 128],
                                        identity=ident[0:16, 0:16])
                S = spool.tile([128, RB, 2, 24], F32, tag="S")
                for r in range(RB):
                    din = epiT[:, r * 16:(r + 1) * 16] \
                        .rearrange("p (t s) -> p s t", s=2)[:, :, :, None] \
                        .to_broadcast([128, 2, T, 3])
                    hin = dh[:, r, None, None, :].to_broadcast([128, 2, T, 3])
                    nc.vector.tensor_tensor(
                        out=S[:, r, :, :].rearrange("p s (t k) -> p s t k", k=3),
                        in0=din, in1=hin, op=mybir.AluOpType.mult)

                # absorb the S producer tick, then two conflict-free 512-record
                # scatter-adds (probe-validated granularity) to round-robin
                # partials so each carries only the partial WAW wait
                nc.gpsimd.dma_start(out=scrap[0:1, 1:2], in_=S[0:1, 0, 0, 0:1])
                Sv = S[:].rearrange("p r s e -> p (r s) e")
                for half in range(2):
                    nc.gpsimd.dma_scatter_add(
                        out_ap=partials[(2 * st + half) % KP][:, 0:24],
                        in_ap=Sv[:, half * 4:(half + 1) * 4, :],
                        idxs_ap=sidx_t[:, st * GCOL + half * (GCOL // 2):
                                       st * GCOL + (half + 1) * (GCOL // 2)],
                        num_idxs=ST,
                        num_idxs_reg=ST,
                        elem_size=24,
                        elem_step=64,
                    )

    nc.finalize()
    return nc


# ---------------------------------------------------------------------------
# host-side sharding / index preparation
# ---------------------------------------------------------------------------

def _wrap16(lin):
    """linear int16 idx list -> [128, len/16] tile layout (8x replicated)."""
    w = lin.reshape(-1, 16).T
    return np.tile(w, (8, 1)).astype(np.int16).copy()


def _prep_core_inputs(bonds_shard, encoded_bf, coords, consts):
    nreal = bonds_shard.shape[0]
    i0g = np.asarray(bonds_shard[:, 0], np.int64)
    i1g = np.asarray(bonds_shard[:, 1], np.int64)
    uniq, inv = np.unique(np.concatenate([i0g, i1g]), return_inverse=True)
    U = uniq.size
    assert U <= UPAD - 8, f"unique atoms {U} exceed UPAD-8"
    i0L = inv[:nreal].astype(np.int32)
    i1L = inv[nreal:].astype(np.int32)

    tabL = np.zeros((UPAD, 128), ml_dtypes.bfloat16)
    tabL[:U] = encoded_bf[uniq]

    # supertile coloring: atom at most once per supertile (over both roles)
    deg = np.bincount(inv, minlength=U)
    edif = np.maximum(deg[i0L], deg[i1L])
    order = np.argsort(-edif, kind="stable")
    fill = np.zeros(NST, np.int32)
    seen = [set() for _ in range(NST)]
    slot_of = np.full(nreal, -1, np.int64)   # global slot index in [0, EC)
    selfloop = np.zeros(nreal, bool)
    for e in order:
        a, b = int(i0L[e]), int(i1L[e])
        if a == b:
            selfloop[e] = True
            # contributes exactly zero (dh == 0); records go to TRASH
            stv = int(np.argmin(fill))
            assert fill[stv] < ST, "no space for self-loop edge"
        else:
            stv, best_fill = -1, ST
            for cand in range(NST):
                if fill[cand] < ST and fill[cand] < best_fill \
                        and a not in seen[cand] and b not in seen[cand]:
                    stv, best_fill = cand, fill[cand]
            assert stv >= 0, "supertile coloring failed"
            seen[stv].add(a)
            seen[stv].add(b)
        slot_of[e] = stv * ST + fill[stv]
        fill[stv] += 1

    # build gather / scatter index lists and cpair
    gidx_lin = np.zeros(NST * 2 * ST, np.int64)       # pad -> atom 0
    sidx_lin = np.full(NST * 2 * ST, TRASH, np.int64)
    cpair_h = np.zeros((128, NST * RB * 6), np.float32)

    slots = slot_of
    stv = slots // ST
    k = slots % ST
    r = k // 128
    part = k % 128
    # gather positions: st*2*ST + k (i0) and st*2*ST + ST + k (i1)
    gidx_lin[stv * 2 * ST + k] = i0L
    gidx_lin[stv * 2 * ST + ST + k] = i1L
    # scatter records: chunk c = 2r+s, position st*2*ST + c*128 + part
    pos0 = stv * 2 * ST + (2 * r) * 128 + part
    pos1 = stv * 2 * ST + (2 * r + 1) * 128 + part
    ok = ~selfloop
    sidx_lin[pos0[ok]] = i0L[ok]
    sidx_lin[pos1[ok]] = i1L[ok]
    # cpair[part, st*24 + r*6 + k]
    col = stv * (RB * 6) + r * 6
    cpair_h[part, col + 0] = coords[i0g, 0]
    cpair_h[part, col + 1] = coords[i0g, 1]
    cpair_h[part, col + 2] = coords[i0g, 2]
    cpair_h[part, col + 3] = coords[i1g, 0]
    cpair_h[part, col + 4] = coords[i1g, 1]
    cpair_h[part, col + 5] = coords[i1g, 2]

    inp = dict(
        tabL=tabL,
        gidx=_wrap16(gidx_lin),
        sidx=_wrap16(sidx_lin),
        cpair=cpair_h,
    )
    inp.update(consts)
    return inp, uniq, U


def _make_consts(W1, b1, b2, b3, W2, W3, W4, b4, t):
    W1 = np.asarray(W1, np.float32)
    b4 = np.asarray(b4, np.float32)
    b4c = np.empty((16, 1), np.float32)
    b4c[0::2, 0] = -0.5 * b4[0]
    b4c[1::2, 0] = 0.5 * b4[1]
    return dict(
        W1a=np.ascontiguousarray(W1[0:128, :]),
        W1b=np.ascontiguousarray(W1[128:256, :]),
        wt=np.ascontiguousarray(W1[256, :].reshape(128, 1)),
        wdl=np.ascontiguousarray(W1[257, :].reshape(1, 128)),
        b1=np.asarray(b1, np.float32).reshape(128, 1).copy(),
        b2=np.asarray(b2, np.float32).reshape(128, 1).copy(),
        b3=np.asarray(b3, np.float32).reshape(128, 1).copy(),
        W2=np.ascontiguousarray(np.asarray(W2, np.float32)),
        W3=np.ascontiguousarray(np.asarray(W3, np.float32)),
        W4=np.ascontiguousarray(np.asarray(W4, np.float32)),
        b4c=b4c,
        tb=np.ascontiguousarray(np.broadcast_to(np.asarray(t, np.float32),
                                                (128, T))),
    )


def _kernel_impl(coords, encoded, t, answer, W1, b1, W2, b2, W3, b3, W4, b4,
                 bonds, trace=False):
    coords = np.asarray(coords, np.float32)
    encoded = np.asarray(encoded, np.float32)
    answer = np.asarray(answer, np.float32)
    bonds = np.asarray(bonds)
    encoded_bf = encoded.astype(ml_dtypes.bfloat16)

    consts = _make_consts(W1, b1, b2, b3, W2, W3, W4, b4, t)

    in_maps, uniqs, us = [], [], []
    for c in range(NCORES):
        shard = bonds[c * EPC:(c + 1) * EPC]
        inp, uniq, U = _prep_core_inputs(shard, encoded_bf, coords, consts)
        in_maps.append(inp)
        uniqs.append(uniq)
        us.append(U)

    nc = build_kernel()
    kw = dict(trace=True, trace_cores=[0]) if trace else {}
    res = run_bass_kernel_spmd(nc, in_maps, core_ids=list(range(NCORES)), **kw)

    out = answer.reshape(N, T * 3).astype(np.float32).copy()
    for c in range(NCORES):
        acc = np.zeros((us[c], 24), np.float32)
        for q in range(KP):
            acc += res.results[c][f"partial{q}"][:us[c], 0:24]
        out[uniqs[c]] += acc
    return out.reshape(N, T, 3), (res.exec_time_ns if trace else None)


def kernel(coords, encoded, t, answer, W1, b1, W2, b2, W3, b3, W4, b4, bonds):
    out, _ = _kernel_impl(coords, encoded, t, answer, W1, b1, W2, b2, W3, b3,
                          W4, b4, bonds)
    return out


def kernel_traced(coords, encoded, t, answer, W1, b1, W2, b2, W3, b3, W4, b4,
                  bonds):
    return _kernel_impl(coords, encoded, t, answer, W1, b1, W2, b2, W3, b3,
                        W4, b4, bonds, trace=True)


if __name__ == "__main__":
    nc = build_kernel()
    print("built ok")
